# revision 26
# baseline (speedup 1.0000x reference)
"""DirectedGraphLayer (GNN message passing) on 8 Trainium2 NeuronCores.

out = relu(x @ W_self + b_self + segment_sum(edge_val * (x@W)[edge_col], edge_row))

v7: replicated-xT, no collective, engine-disjoint pipelined phases.
  - Every core receives the FULL pre-transposed x (26 MB bf16, slice-major
    blocks) and computes the complete x_trans = x@W table (message halves
    only, free-64 matmuls) into two DRAM half-tables (source tiles [0,200) /
    [200,400)), row-permuted as row = p*TH + t_local so stores are full-rate
    and gather indices fit int16.  No AllGather: the 13 MB collective (252 us
    under the collective cost floor) is replaced by ~44 us of redundant local
    compute that overlaps the edge pipeline.
  - Edges (dest-sharded) are classed by source half: per-edge row gathers +
    one-hot-selector segment matmuls for half 0 start as soon as half-table 0
    lands (~45 us); half 1 overlaps the rest of the table build.
  - Engine-disjoint roles (engine queues are in-order): SP = xT loads, h0-odd
    table stores, out stores; Pool = 4 of the phase-1a xT loads (as
    sequential-index dma_gathers) + all 22 edge gathers on 4 SWDGE queues;
    DVE = phase-1a PSUM drains, selectors, h0->accumulator copies; Act =
    input loads, phase-1b drains, table stores, relu; PE = transforms,
    segment matmuls, bias (e0 x biasrow), self term, and the h1 accumulator
    read-back (identity matmul of the bf16 accumulator).
  - Dest nodes are bin-packed (greedy by per-class in-degree, then bins
    sorted by class-0 load and regrouped 8-per-tile) so per-(tile, class)
    edge blocks across cores pad to ~1024 rows (4% padding).
  - SWDGE queue_num is patched post-compile from each gather's assigned
    DMASW sem lane (lane % NQ) -- the tile scheduler may reorder gathers, and
    each DMASW sem lane is locked to a single queue.
"""

import heapq

import numpy as np

import concourse.bacc as bacc
import concourse.mybir as mybir
import concourse.tile as tile
from concourse.bass_utils import run_bass_kernel_spmd

NCORES = 8
FIN = 128
FOUT = 64
B = 2
GROUPS = [3, 5, 5, 5, 5, 5, 5, 5, 5, 5, 2]   # dest tiles per gather (per class)
NQ = 4         # SWDGE queues
NSLICES = 20   # phase-1 slices over all 400 source tiles

BF16 = mybir.dt.bfloat16
NP_BF16 = mybir.dt.np(BF16)


def _plan(N, edge_row, edge_col, edge_val):
    """Host-side planning: dest bin-packing + edge partitioning.

    cls = source half by tile (t_g < TH).  Table row = p_g*TH + t_local.
    """
    npc = -(-N // NCORES)
    tiles = -(-npc // 128)
    if tiles < sum(GROUPS):
        tiles = sum(GROUPS)
    assert tiles == sum(GROUPS)
    npc_pad = tiles * 128
    ntt = NCORES * tiles
    TH = ntt // 2
    assert 128 * TH - 1 <= 32767

    # source mapping (canonical padded node -> table row/half)
    scr = edge_col // npc
    sloc = edge_col % npc
    t_g = scr * tiles + sloc // 128
    p_g = sloc % 128
    cls = (t_g >= TH).astype(np.int64)
    src_rel = (p_g * TH + (t_g - cls * TH)).astype(np.int16)

    # dest bin-packing: balance per-(bin, cls) edge counts across cores
    d0 = np.bincount(edge_row[cls == 0], minlength=N)
    d1 = np.bincount(edge_row[cls == 1], minlength=N)
    nbins = NCORES * tiles
    node_order = np.argsort(-(d0 + d1), kind="stable")
    heap = [(0, b) for b in range(nbins)]
    heapq.heapify(heap)
    used = np.zeros(nbins, dtype=np.int64)
    bin_of = np.empty(N, dtype=np.int64)
    slot_of = np.empty(N, dtype=np.int64)
    s0 = np.zeros(nbins, dtype=np.int64)
    for n in node_order:
        while True:
            load, b = heapq.heappop(heap)
            if used[b] < 128:
                break
        bin_of[n] = b
        slot_of[n] = used[b]
        used[b] += 1
        s0[b] += int(d0[n])
        heapq.heappush(heap, (load + int(d0[n]) + int(d1[n]), b))
    # relabel bins: sorted by s0, consecutive groups of NCORES share a tile ->
    # per-(tile, cls) maxima over cores stay tight (s1 anti-correlates with s0)
    rank_of = np.empty(nbins, dtype=np.int64)
    rank_of[np.argsort(s0, kind="stable")] = np.arange(nbins)
    c_of = rank_of[bin_of] % NCORES
    t_of = rank_of[bin_of] // NCORES

    dcore = c_of[edge_row]
    t_of_e = t_of[edge_row]
    slot = slot_of[edge_row]

    key = (dcore * tiles + t_of_e) * 2 + cls
    order = np.argsort(key, kind="stable")
    key_s = key[order]
    nkeys = NCORES * tiles * 2
    counts = np.bincount(key_s, minlength=nkeys).reshape(NCORES, tiles, 2)

    pad = counts.max(axis=0)                       # (tiles, 2)
    pad = ((pad + 127) // 128) * 128
    pad = np.maximum(pad, 128)

    per_core_edges = int(pad.sum())
    nchunks = per_core_edges // 128
    s_total = per_core_edges // 16
    s_total = ((s_total + 127) // 128) * 128   # idx gather needs 256B-mult rows

    # layout: per group g: [t..c0][t..c1]
    block_off = np.zeros((tiles, 2), dtype=np.int64)
    run = 0
    gathers = []    # (cls, n, chunk_base) ordered [g0c0, g0c1, g1c0, ...]
    gstart = np.concatenate([[0], np.cumsum(GROUPS)])
    for g in range(len(GROUPS)):
        ts = list(range(gstart[g], gstart[g + 1]))
        for h in (0, 1):
            n = int(sum(pad[t, h] for t in ts))
            gathers.append((h, n, run // 128))
            for t in ts:
                block_off[t, h] = run
                run += int(pad[t, h])
    assert run == per_core_edges

    first_of_key = np.zeros(nkeys + 1, dtype=np.int64)
    np.cumsum(np.bincount(key_s, minlength=nkeys), out=first_of_key[1:])
    rank = np.arange(len(key_s)) - first_of_key[key_s]
    c_s = dcore[order]
    t_s = t_of_e[order]
    cl_s = cls[order]
    pos = block_off[t_s, cl_s] + rank

    idx_vals = np.zeros((NCORES, per_core_edges), dtype=np.int16)
    slot_arr = np.zeros((NCORES, per_core_edges), dtype=np.float32)
    val_arr = np.zeros((NCORES, per_core_edges), dtype=np.float32)
    idx_vals[c_s, pos] = src_rel[order]
    slot_arr[c_s, pos] = slot[order].astype(np.float32)
    val_arr[c_s, pos] = edge_val[order]

    slot_t = slot_arr.reshape(NCORES, nchunks, 128).transpose(0, 2, 1).copy()
    val_t = val_arr.reshape(NCORES, nchunks, 128).transpose(0, 2, 1).copy()

    idx_t = np.zeros((NCORES, 128, s_total), dtype=np.int16)
    for (_h, n, cb) in gathers:
        if n == 0:
            continue
        blk = idx_vals[:, cb * 128: cb * 128 + n]
        wrapped = blk.reshape(NCORES, n // 16, 16).transpose(0, 2, 1)
        idx_t[:, :, cb * 8: cb * 8 + n // 16] = np.tile(wrapped, (1, 8, 1))

    return dict(
        npc=npc, tiles=tiles, npc_pad=npc_pad, ntt=ntt, TH=TH,
        nchunks=nchunks, s_total=s_total, gathers=gathers,
        block_off=block_off, pad=pad,
        c_of=c_of, t_of=t_of, slot_of=slot_of,
        idx_t=idx_t, slot_t=slot_t, val_t=val_t,
    )


def _build(plan, repeat=1):
    tiles, npc_pad = plan["tiles"], plan["npc_pad"]
    ntt, TH = plan["ntt"], plan["TH"]
    nchunks, s_total = plan["nchunks"], plan["s_total"]
    f32 = mybir.dt.float32

    nc = bacc.Bacc("TRN2", target_bir_lowering=False,
                   num_devices=NCORES, num_swdge_queues=NQ)
    SLW = ntt * B * 128 // NSLICES     # elems per fin-row per slice block
    xT_in = nc.dram_tensor("xT", [NSLICES * 128, SLW], BF16, kind="ExternalInput")
    ox_in = nc.dram_tensor("ox", [128, tiles * B * 128], BF16, kind="ExternalInput")
    w_in = nc.dram_tensor("w", [128, FOUT], BF16, kind="ExternalInput")
    ws0_in = nc.dram_tensor("ws0", [128, 128], BF16, kind="ExternalInput")
    ws1_in = nc.dram_tensor("ws1", [128, 128], BF16, kind="ExternalInput")
    ident_in = nc.dram_tensor("ident", [128, 128], BF16, kind="ExternalInput")
    e0_in = nc.dram_tensor("e0", [128, 128], BF16, kind="ExternalInput")
    bias_in = nc.dram_tensor("bias", [128, 128], BF16, kind="ExternalInput")
    iota_in = nc.dram_tensor("iota", [128, 128], BF16, kind="ExternalInput")
    idx_in = nc.dram_tensor("idx", [128, s_total], mybir.dt.int16, kind="ExternalInput")
    seqidx_in = nc.dram_tensor("seqidx", [128, 8], mybir.dt.int16, kind="ExternalInput")
    slot_in = nc.dram_tensor("slot", [128, nchunks], f32, kind="ExternalInput")
    val_in = nc.dram_tensor("val", [128, nchunks], f32, kind="ExternalInput")
    out_d = nc.dram_tensor("out", [npc_pad, 128], f32, kind="ExternalOutput")

    qn = [0]

    def next_q():
        q = qn[0]
        qn[0] = (qn[0] + 1) % NQ
        return q

    t_per_s = ntt // NSLICES
    assert TH % t_per_s == 0
    mm_per_s = t_per_s * B
    assert mm_per_s % 8 == 0          # free-64 matmuls per full PSUM bank

    with tile.TileContext(nc) as tc:
        with (
            tc.tile_pool(name="persist", bufs=1) as pp,
            tc.tile_pool(name="dram", bufs=1, space="DRAM") as dram,
        ):
            idx = pp.tile([128, s_total], mybir.dt.int16)
            with tc.tile_wait_until(0.03):
                nc.scalar.dma_start(idx[:], idx_in.ap())
            seqidx = pp.tile([128, 8], mybir.dt.int16)
            nc.scalar.dma_start(seqidx[:], seqidx_in.ap())
            slot = pp.tile([128, nchunks], f32)
            nc.scalar.dma_start(slot[:], slot_in.ap())
            val = pp.tile([128, nchunks], f32)
            nc.scalar.dma_start(val[:], val_in.ap())
            iota = pp.tile([128, 128], BF16)
            nc.scalar.dma_start(iota[:], iota_in.ap())
            w = pp.tile([128, FOUT], BF16)
            nc.scalar.dma_start(w[:], w_in.ap())
            ws0 = pp.tile([128, 128], BF16)
            nc.scalar.dma_start(ws0[:], ws0_in.ap())
            ws1 = pp.tile([128, 128], BF16)
            nc.scalar.dma_start(ws1[:], ws1_in.ap())
            ident = pp.tile([128, 128], BF16)
            nc.scalar.dma_start(ident[:], ident_in.ap())
            e0 = pp.tile([128, 128], BF16)
            nc.scalar.dma_start(e0[:], e0_in.ap())
            bias = pp.tile([128, 128], BF16)
            nc.scalar.dma_start(bias[:], bias_in.ap())
            own_x = pp.tile([128, tiles * B * 128], BF16)
            acc = pp.tile([128, npc_pad], BF16)   # bias + self + h0 partials

            def one_pass():
                tab0 = dram.tile([TH * 128, 128], BF16, tag="tab0")
                tab1 = dram.tile([TH * 128, 128], BF16, tag="tab1")
                tabs = [tab0, tab1]
                with (
                    tc.tile_pool(name="ph1", bufs=3) as p1,
                    tc.tile_pool(name="mmps", bufs=3, space="PSUM") as mmps,
                    tc.tile_pool(name="sel", bufs=16) as selp,
                    tc.tile_pool(name="ps2", bufs=5, space="PSUM") as ps2,
                    tc.tile_pool(name="gat", bufs=2) as p2,
                    tc.tile_pool(name="outp", bufs=4) as outp,
                ):
                    def ph1_half(hh):
                        deferred = []
                        nsl = NSLICES // 2
                        oxw = tiles * B * 128 // nsl
                        pool_slices = (1, 3, 5, 7) if hh == 0 else ()
                        for s in range(hh * nsl, (hh + 1) * nsl):
                            t0 = s * t_per_s
                            xts = p1.tile([128, t_per_s * B * 128], BF16, tag="xts")
                            blk = xT_in[s * 128:(s + 1) * 128, :]
                            if s in pool_slices:
                                nc.gpsimd.dma_gather(
                                    xts[:].rearrange("p (a c) -> p a c", a=1),
                                    blk, seqidx[:], 128, 128, SLW,
                                    single_packet=False, queue_num=next_q())
                            else:
                                nc.sync.dma_start(xts[:], blk)
                            stage = p1.tile([128, t_per_s * 128], BF16,
                                            tag=f"stage{hh}",
                                            bufs=(3 if hh == 0 else 6))
                            bank = None
                            nbanks = mm_per_s // 8
                            for mi in range(mm_per_s):
                                j = mi % 8
                                if j == 0:
                                    bank = mmps.tile([128, 512], f32, tag="bank")
                                nc.tensor.matmul(
                                    bank[:, j * 64:(j + 1) * 64],
                                    xts[:, mi * 128:(mi + 1) * 128], w[:],
                                    start=True, stop=True)
                                if j == 7:
                                    dst = stage[:, (mi - 7) * 64:(mi + 1) * 64]
                                    if hh == 0 and (mi // 8) % 5 < 3:
                                        nc.vector.tensor_copy(dst, bank[:])
                                    else:
                                        nc.scalar.copy(dst, bank[:])
                            tl0 = t0 - hh * TH
                            dstap = tabs[hh][:].rearrange(
                                "(p t) c -> p t c", t=TH)[:, tl0: tl0 + t_per_s, :]
                            srcap = stage[:].rearrange("p (t c) -> p t c", c=128)
                            if hh == 0:
                                if s % 2 == 0:
                                    nc.scalar.dma_start(dstap, srcap)
                                else:
                                    nc.sync.dma_start(dstap, srcap)
                            else:
                                deferred.append((dstap, srcap))
                                if s == hh * nsl + 4:
                                    for (d2, s2) in deferred:
                                        nc.scalar.dma_start(d2, s2)
                                    deferred = []
                        for i, (dstap, srcap) in enumerate(deferred):
                            (nc.sync if i % 2 == 0 else nc.scalar).dma_start(
                                dstap, srcap)
                        if hh == 1:
                            for si in range(nsl):
                                with tc.tile_wait_until(0.075):
                                    nc.scalar.dma_start(
                                        own_x[:, si * oxw:(si + 1) * oxw],
                                        ox_in[:, si * oxw:(si + 1) * oxw])

                    def seg_round(h):
                        gstart = [0]
                        for gg in GROUPS:
                            gstart.append(gstart[-1] + gg)
                        for g in range(len(GROUPS)):
                            (hh, n, cb) = plan["gathers"][2 * g + h]
                            assert hh == h
                            gth = p2.tile([128, n // 128, 128], BF16, tag=f"g{h}",
                                          bufs=(2 if h == 0 else 3))
                            nc.gpsimd.dma_gather(
                                gth[:], tabs[h][:],
                                idx[:, cb * 8: cb * 8 + n // 16],
                                n, n, 128, elem_step=128, single_packet=False,
                                queue_num=next_q())
                            for ti in range(GROUPS[g]):
                                t = gstart[g] + ti
                                ps = ps2.tile([128, 128], f32, tag="ps")
                                if h == 0:
                                    nc.tensor.matmul(
                                        ps[:], e0[:], bias[:],
                                        start=True, stop=False)
                                if h == 1:
                                    nc.tensor.matmul(
                                        ps[:], ident[:],
                                        acc[:, t * 128:(t + 1) * 128],
                                        start=True, stop=False)
                                    nc.tensor.matmul(
                                        ps[:],
                                        own_x[:, (t * B) * 128:(t * B + 1) * 128],
                                        ws0[:], start=False, stop=False)
                                    nc.tensor.matmul(
                                        ps[:],
                                        own_x[:, (t * B + 1) * 128:(t * B + 2) * 128],
                                        ws1[:], start=False, stop=False)
                                c0 = plan["block_off"][t, h] // 128
                                cn = plan["pad"][t, h] // 128
                                for c in range(cn):
                                    lc = c0 + c
                                    sel = selp.tile([128, 128], BF16, tag="sel")
                                    nc.vector.tensor_scalar(
                                        sel[:], iota[:], slot[:, lc:lc + 1],
                                        val[:, lc:lc + 1],
                                        mybir.AluOpType.is_equal,
                                        mybir.AluOpType.mult)
                                    nc.tensor.matmul(
                                        ps[:], sel[:], gth[:, lc - cb, :],
                                        start=False, stop=(c == cn - 1))
                                if h == 0:
                                    nc.vector.tensor_copy(
                                        acc[:, t * 128:(t + 1) * 128], ps[:])
                                else:
                                    ot = outp.tile([128, 128], f32, tag="ot")
                                    nc.scalar.activation(
                                        ot[:], ps[:],
                                        mybir.ActivationFunctionType.Relu)
                                    nc.sync.dma_start(
                                        out_d[t * 128:(t + 1) * 128, :], ot[:])

                    ph1_half(0)
                    ph1_half(1)
                    seg_round(0)
                    seg_round(1)

            for _rep in range(repeat):
                one_pass()
    nc.compile()
    # SWDGE queue <-> DMASW-sem-lane consistency: the tile scheduler may
    # permute gather order, and each DMASW sem lane is locked to one queue.
    # Derive the queue from the lane the framework actually assigned.
    import re
    for bl in nc.m.functions[0].blocks:
        for inst in bl.instructions:
            if type(inst).__name__ == "InstDMAGatherAnt" and inst.sync_info:
                for u in inst.sync_info.on_update:
                    m = re.match(r"DMASW(\d+)_", getattr(u, "ant_name", "") or "")
                    if m:
                        inst.queue_num = int(m.group(1)) % NQ
                        break
    return nc


def _prepare(x, W, W_self, b_self, edge_row, edge_col, edge_val, repeat=1):
    Bx, N, fin = x.shape
    assert Bx == B and fin == FIN and W.shape == (FIN, FOUT)
    plan = _plan(N, edge_row.astype(np.int64), edge_col.astype(np.int64),
                 edge_val.astype(np.float32))
    npc, npc_pad, tiles, ntt = plan["npc"], plan["npc_pad"], plan["tiles"], plan["ntt"]

    # xT: canonical source order, col = ((t*B + b)*128 + p)
    xpad = np.zeros((B, ntt * 128, FIN), dtype=np.float32)
    for k in range(NCORES):
        lo = k * npc
        hi = min(N, lo + npc)
        xpad[:, k * npc_pad: k * npc_pad + hi - lo] = x[:, lo:hi]
    xT = xpad.reshape(B, ntt, 128, FIN).transpose(3, 1, 0, 2) \
             .reshape(FIN, ntt * B * 128).astype(NP_BF16)
    # slice-major blocks: row = s*128 + fin
    xT = np.ascontiguousarray(
        xT.reshape(FIN, NSLICES, ntt * B * 128 // NSLICES).transpose(1, 0, 2)
    ).reshape(NSLICES * FIN, ntt * B * 128 // NSLICES)

    # own_x: per core, dest-permuted raw x, col = ((t*B + b)*128 + slot)
    c_of, t_of, slot_of = plan["c_of"], plan["t_of"], plan["slot_of"]
    node_at = np.full((NCORES, tiles, 128), -1, dtype=np.int64)
    node_at[c_of, t_of, slot_of] = np.arange(N)
    oxs = []
    for k in range(NCORES):
        nk = node_at[k].reshape(tiles * 128)
        mask = nk >= 0
        xk = x[:, np.clip(nk, 0, None)] * mask[None, :, None]   # (B, tiles*128, FIN)
        ox = xk.reshape(B, tiles, 128, FIN).transpose(3, 1, 0, 2) \
               .reshape(FIN, tiles * B * 128).astype(NP_BF16)
        oxs.append(ox)

    wq = W.astype(NP_BF16)
    z = np.zeros_like(W_self)
    ws0q = np.concatenate([W_self, z], axis=1).astype(NP_BF16)
    ws1q = np.concatenate([z, W_self], axis=1).astype(NP_BF16)
    identq = np.eye(128, dtype=np.float32).astype(NP_BF16)
    e0q = np.zeros((128, 128), dtype=np.float32)
    e0q[0, :] = 1.0
    e0q = e0q.astype(NP_BF16)
    bias128 = np.zeros((128, 128), dtype=np.float32)
    bias128[0, :] = np.concatenate([b_self, b_self])
    bias128 = bias128.astype(NP_BF16)
    iota = np.tile(np.arange(128, dtype=np.float32)[None, :], (128, 1)).astype(NP_BF16)
    seqidx = np.tile(np.arange(128, dtype=np.int16).reshape(8, 16).T, (8, 1))

    in_maps = []
    for k in range(NCORES):
        in_maps.append({
            "xT": xT, "ox": oxs[k], "w": wq, "ws0": ws0q, "ws1": ws1q,
            "ident": identq, "e0": e0q, "bias": bias128, "iota": iota,
            "seqidx": seqidx,
            "idx": plan["idx_t"][k], "slot": plan["slot_t"][k],
            "val": plan["val_t"][k],
        })

    def assemble(results):
        outs = np.stack([results[k]["out"] for k in range(NCORES)])  # (C, npc_pad, 128)
        rows = outs[c_of, t_of * 128 + slot_of]                      # (N, 128)
        return np.ascontiguousarray(
            rows.reshape(N, B, FOUT).transpose(1, 0, 2))

    nc = _build(plan, repeat=repeat)
    return nc, in_maps, assemble


def kernel(x, W, W_self, b_self, edge_row, edge_col, edge_val):
    nc, in_maps, assemble = _prepare(
        np.asarray(x), np.asarray(W), np.asarray(W_self), np.asarray(b_self),
        np.asarray(edge_row), np.asarray(edge_col), np.asarray(edge_val),
    )
    res = run_bass_kernel_spmd(nc, in_maps, core_ids=list(range(NCORES)))
    return assemble(res.results)
